# revision 56
# baseline (speedup 1.0000x reference)
"""GIN-style 3-layer GNN encoder on 8 Trainium2 NeuronCores (Bass/Tile).

Reference computation (fp32):
    h = x @ W_in.T + b_in                                  [50000, 96]
    for l in 0..2:
        agg = segment_sum(h[src], dst, N)                  [50000, 96]
        h = (h + agg) @ W_layers[l].T + b_layers[l]
    out = concat([h0..h3], 1) @ W_out.T + b_out            [50000, 128]

Distribution: nodes are partitioned across the 8 cores (6250/core) via a
host-side balancing permutation; each edge is owned by the core that owns
its dst node.  Each layer the updated node features are AllGathered into
two replicated row-major fp16 tables h_fullA/h_fullB (first/second half
of every core's node range, 25000 x 256B rows each, one Shared-space
pair per state so collectives write once and never alias) — the split
halves the AllGather latency on the critical path and keeps gather
indices < 32768 (int16).

Per-core segment sum: a core's node range is split into 49 windows of 128
nodes.  Every window has a fixed number of 128-edge tiles (T_a tiles with
src in half A, T_b in half B; the balancing permutation equalizes
per-window per-class edge counts so the fixed tile counts are tight).
Edge features are fetched with gpsimd dma_gather (fp16 256B rows, 1024
idxs per instruction, round-robin over the 4 SWDGE queues).  For each
window the one-hot onehot[e, j, t] = (j == dst_local[e, t]) is built on
DVE (layout [128, WIN, T] keeps every operand's last dim stride-1 so the
2x DVE perf mode engages), and the PE accumulates
    psum[96, 128] += gathered_tile[128e, 96].T @ onehot[:, :, t]
which is aggT for the window.

The whole layer is chunk-pipelined: each chunk of 4 windows (512 nodes =
one PSUM bank) flows gathers -> onehot/agg -> +h (DVE) -> layer matmul ->
bias -> PE transpose to the row-major fp16 shard.  bounceA DMAs fire
mid-layer (after window 24) and AllGather-A for the next table is emitted
mid-gather-stream (chunk 9) so its flight overlaps the rest of the layer;
AllGather-B fires at layer end (a collective's sequencer wait only blocks
until its input is ready — the flight itself runs async on the CC cores).
The final output projection is interleaved into layer 2's chunk loop.
"""
import sys

sys.path.insert(0, "/opt/trn_rl_repo")

import numpy as np

N_NODES = 50000
N_EDGES = 800000
IN_DIM = 128
HID = 96
OUT_DIM = 128
N_LAYERS = 3
N_CORES = 8
NPC = N_NODES // N_CORES          # 6250 nodes per core
WIN = 128                         # window width (nodes)
NW = (NPC + WIN - 1) // WIN       # 49 windows per core (last = 106 nodes)
HALF = NPC // 2                   # 3125: per-core A/B split
CLS = N_CORES * HALF              # 25000: A-class size
AW = HALF // WIN                  # 24 full-A windows per core
REM_A = HALF - AW * WIN           # 53 A-slots in window 24
CHUNK_W = 4                       # windows per chunk (= 512 nodes = 1 bank)
GT = 8                            # tiles per dma_gather (1024 idxs)
CW_N = CHUNK_W * WIN              # 512: node-chunk for dense matmuls
BOUNCE_A_CHUNK = (AW * WIN + REM_A - 1) // CW_N   # chunk whose transposes
                                                  # complete the A half (6)

_cache = {}


def _balance_nodes(src0, dst0):
    """Permute node ids so per-(core,window) A/B edge counts are even.

    A node's A/B class (which replicated gather table its row lives in) is
    frozen to its OLD id (< CLS -> A); the permutation only moves nodes
    within their class region, so per-node (deg_a, deg_b) are fixed and a
    greedy 2-D balance over the 392 (core, window) bins makes the uniform
    tile counts T_a/T_b tight.  Returns perm (old id -> new id).
    """
    deg_a = np.bincount(dst0[src0 < CLS], minlength=N_NODES).astype(np.int64)
    deg_b = np.bincount(dst0[src0 >= CLS], minlength=N_NODES).astype(np.int64)
    nbins = N_CORES * NW
    base = np.empty(nbins, np.int64)
    cap = np.empty(nbins, np.int64)
    for b in range(nbins):
        c, w = divmod(b, NW)
        base[b] = c * NPC + w * WIN
        cap[b] = min(WIN, NPC - w * WIN)
    woff = base % NPC
    q_a = np.clip(HALF - woff, 0, cap)   # A slots = first q_a of the window
    q_b = cap - q_a

    mu_a = max(1.0, deg_a.sum() / nbins)
    mu_b = max(1.0, deg_b.sum() / nbins)
    order = np.argsort(-(deg_a + deg_b), kind="stable")
    a_load = np.zeros(nbins)
    b_load = np.zeros(nbins)
    a_left = q_a.copy()
    b_left = q_b.copy()
    a_pos = np.zeros(nbins, np.int64)
    b_pos = q_a.copy()
    perm = np.empty(N_NODES, np.int64)
    for n in order:
        phi = np.maximum((a_load + deg_a[n]) / mu_a,
                         (b_load + deg_b[n]) / mu_b)
        if n < CLS:
            phi = np.where(a_left > 0, phi, np.inf)
            b_ = int(np.argmin(phi))
            perm[n] = base[b_] + a_pos[b_]
            a_pos[b_] += 1
            a_left[b_] -= 1
        else:
            phi = np.where(b_left > 0, phi, np.inf)
            b_ = int(np.argmin(phi))
            perm[n] = base[b_] + b_pos[b_]
            b_pos[b_] += 1
            b_left[b_] -= 1
        a_load[b_] += deg_a[n]
        b_load[b_] += deg_b[n]
    return perm


def _prep(edge_index):
    """Host-side edge bucketing -> per-core gather index / dst tables."""
    src0 = edge_index[0].astype(np.int64)
    dst0 = edge_index[1].astype(np.int64)
    perm = _balance_nodes(src0, dst0)
    src = perm[src0]
    dst = perm[dst0]
    core = dst // NPC
    din = dst % NPC
    w = din // WIN
    dstl = din % WIN
    s_in = src % NPC
    c_src = src // NPC
    is_b = (s_in >= HALF).astype(np.int64)
    pos = np.where(is_b == 0, c_src * HALF + s_in,
                   c_src * HALF + s_in - HALF)  # < 25000, int16-safe

    key = (core * NW + w) * 2 + is_b
    order = np.argsort(key, kind="stable")
    s_pos = pos[order]
    s_dstl = dstl[order]
    s_key = key[order]
    s_b = is_b[order]

    counts = np.bincount(key, minlength=N_CORES * NW * 2)
    T_a = max(1, int(-(-counts.reshape(-1, 2)[:, 0].max() // 128)))
    T_b = max(1, int(-(-counts.reshape(-1, 2)[:, 1].max() // 128)))
    T = T_a + T_b

    starts = np.zeros(N_CORES * NW * 2, np.int64)
    starts[1:] = np.cumsum(counts)[:-1]
    rank = np.arange(len(s_key)) - starts[s_key]

    c_arr = s_key // (2 * NW)
    w_arr = (s_key // 2) % NW

    idx_a = np.zeros((N_CORES, NW, T_a * 128), np.int16)
    idx_b = np.zeros((N_CORES, NW, T_b * 128), np.int16)
    dstl_arr = np.full((N_CORES, NW, T, 128), -1.0, np.float16)

    a_m = s_b == 0
    flat = (c_arr[a_m] * NW + w_arr[a_m]) * (T_a * 128) + rank[a_m]
    idx_a.reshape(-1)[flat] = s_pos[a_m].astype(np.int16)
    t_g = rank[a_m] // 128
    e_g = rank[a_m] % 128
    flat = ((c_arr[a_m] * NW + w_arr[a_m]) * T + t_g) * 128 + e_g
    dstl_arr.reshape(-1)[flat] = s_dstl[a_m].astype(np.float16)

    b_m = ~a_m
    flat = (c_arr[b_m] * NW + w_arr[b_m]) * (T_b * 128) + rank[b_m]
    idx_b.reshape(-1)[flat] = s_pos[b_m].astype(np.int16)
    t_g = rank[b_m] // 128 + T_a
    e_g = rank[b_m] % 128
    flat = ((c_arr[b_m] * NW + w_arr[b_m]) * T + t_g) * 128 + e_g
    dstl_arr.reshape(-1)[flat] = s_dstl[b_m].astype(np.float16)

    def wrap(vals):  # [NW*Tc*128] -> [128, NW*Tc*8] int16 wrapped+replicated
        v = vals.reshape(-1, 16).T
        return np.tile(v, (8, 1)).copy()

    idx_a_w = np.stack([wrap(idx_a[c].reshape(-1)) for c in range(N_CORES)])
    idx_b_w = np.stack([wrap(idx_b[c].reshape(-1)) for c in range(N_CORES)])
    dstloc = np.ascontiguousarray(dstl_arr.transpose(0, 3, 1, 2))  # [C,128,NW,T]
    return idx_a_w, idx_b_w, dstloc, T_a, T_b, perm


def _build(T_a, T_b):
    from concourse import bacc, tile, mybir, library_config

    dt = mybir.dt
    T = T_a + T_b
    nc = bacc.Bacc("TRN2", target_bir_lowering=False, debug=False,
                   num_devices=N_CORES, num_swdge_queues=4,
                   dynamic_dma_scratch_size=49152)

    # ---- I/O ----
    xT_in = nc.dram_tensor("xT", [IN_DIM, NPC], dt.float32, kind="ExternalInput")
    w_inT_in = nc.dram_tensor("w_inT", [IN_DIM, HID], dt.float32,
                              kind="ExternalInput")
    b_in_in = nc.dram_tensor("b_in", [HID, 1], dt.float32, kind="ExternalInput")
    w_lT_in = nc.dram_tensor("w_lT", [N_LAYERS, HID, HID], dt.float32,
                             kind="ExternalInput")
    b_l_in = nc.dram_tensor("b_l", [N_LAYERS, HID, 1], dt.float32,
                            kind="ExternalInput")
    w_out4_in = nc.dram_tensor("w_out4", [N_LAYERS + 1, HID, OUT_DIM],
                               dt.float16, kind="ExternalInput")
    b_out_in = nc.dram_tensor("b_out", [OUT_DIM, 1], dt.float32,
                              kind="ExternalInput")
    iota_in = nc.dram_tensor("iota_wt", [128, WIN, T], dt.float16,
                             kind="ExternalInput")
    id96_in = nc.dram_tensor("id96", [HID, HID], dt.float16,
                             kind="ExternalInput")
    id128_in = nc.dram_tensor("id128", [128, 128], dt.float32,
                              kind="ExternalInput")
    idx_a_in = nc.dram_tensor("idx_a", [128, NW * T_a * 8], dt.int16,
                              kind="ExternalInput")
    idx_b_in = nc.dram_tensor("idx_b", [128, NW * T_b * 8], dt.int16,
                              kind="ExternalInput")
    dstloc_in = nc.dram_tensor("dstloc", [128, NW, T], dt.float16,
                               kind="ExternalInput")
    out_ext = nc.dram_tensor("out", [NPC, OUT_DIM], dt.float32,
                             kind="ExternalOutput")

    f32, f32r, f16 = dt.float32, dt.float32r, dt.float16

    with tile.TileContext(nc, num_cores=N_CORES) as tc:
        nc.gpsimd.load_library(library_config.mlp)
        with tc.tile_pool(name="persist", bufs=1) as pp, \
             tc.tile_pool(name="xpool", bufs=2) as xpool, \
             tc.tile_pool(name="hp", bufs=3) as hp_pool, \
             tc.tile_pool(name="ga", bufs=3) as ga_pool, \
             tc.tile_pool(name="gb", bufs=4) as gb_pool, \
             tc.tile_pool(name="oh", bufs=3) as oh_pool, \
             tc.tile_pool(name="otile", bufs=2) as ot_pool, \
             tc.tile_pool(name="ps_agg", bufs=4, space="PSUM") as ps_agg, \
             tc.tile_pool(name="ps_big", bufs=2, space="PSUM") as ps_big, \
             tc.tile_pool(name="ps_tr", bufs=2, space="PSUM") as ps_tr, \
             tc.tile_pool(name="dram", bufs=1, space="DRAM") as dram:

            def load(name, shape, dtype, src_ap):
                t = pp.tile(shape, dtype, name=name)
                nc.sync.dma_start(out=t[:], in_=src_ap)
                return t

            w_inT = load("w_inT", [IN_DIM, HID], f32r, w_inT_in[:].bitcast(f32r))
            b_in = load("b_in", [HID, 1], f32, b_in_in[:])
            w_lT = [load(f"w_lT{l}", [HID, HID], f32r, w_lT_in[l].bitcast(f32r))
                    for l in range(N_LAYERS)]
            b_l = [load(f"b_l{l}", [HID, 1], f32, b_l_in[l])
                   for l in range(N_LAYERS)]
            w_out4 = [load(f"w_out4_{s}", [HID, OUT_DIM], f16, w_out4_in[s])
                      for s in range(N_LAYERS + 1)]
            b_out = load("b_out", [OUT_DIM, 1], f32, b_out_in[:])
            iota_wt = load("iota_wt", [128, WIN, T], f16, iota_in[:])
            id96 = load("id96", [HID, HID], f16, id96_in[:])
            id128 = load("id128", [128, 128], f32, id128_in[:])
            idx_a = load("idx_a", [128, NW * T_a * 8], dt.int16, idx_a_in[:])
            idx_b = load("idx_b", [128, NW * T_b * 8], dt.int16, idx_b_in[:])
            dstloc = load("dstloc", [128, NW, T], f16, dstloc_in[:])

            h_state = [pp.tile([HID, NPC], f16, name=f"h{s}")
                       for s in range(N_LAYERS + 1)]
            rm_buf = pp.tile([128, NW, 128], f16, name="rm_buf")

            # double-buffered replicated tables: state s lives in buf s%2
            h_fullA = [dram.tile([CLS, 128], f16, name=f"h_fullA{i}",
                                 addr_space="Shared")
                       for i in range(N_LAYERS)]
            h_fullB = [dram.tile([N_NODES - CLS, 128], f16, name=f"h_fullB{i}",
                                 addr_space="Shared")
                       for i in range(N_LAYERS)]
            bounceA = dram.tile([HALF, 128], f16)
            bounceB = dram.tile([NPC - HALF, 128], f16)

            w_chunks = [(c0, min(CHUNK_W, NW - c0))
                        for c0 in range(0, NW, CHUNK_W)]

            def transpose_windows(s, w0, w1):
                for t in range(w0, w1):
                    n0 = t * 128
                    tn = min(128, NPC - n0)
                    pst = ps_tr.tile([128, HID], f16, name="pst")
                    nc.tensor.transpose(pst[:tn, :],
                                        h_state[s][:, n0:n0 + tn], id96[:])
                    nc.scalar.copy(rm_buf[:tn, t, 0:HID], pst[:tn, :])

            def bounce_a():
                nc.sync.dma_start(
                    out=bounceA[0:AW * 128, :].rearrange(
                        "(t p) d -> p t d", p=128),
                    in_=rm_buf[:, 0:AW, :])
                nc.sync.dma_start(out=bounceA[AW * 128:HALF, :],
                                  in_=rm_buf[0:REM_A, AW, :])

            def bounce_b():
                nc.sync.dma_start(out=bounceB[0:WIN - REM_A, :],
                                  in_=rm_buf[REM_A:WIN, AW, :])
                nb_full = NW - AW - 2   # full windows AW+1 .. NW-2
                o0 = WIN - REM_A
                nc.sync.dma_start(
                    out=bounceB[o0:o0 + nb_full * 128, :].rearrange(
                        "(t p) d -> p t d", p=128),
                    in_=rm_buf[:, AW + 1:NW - 1, :])
                o1 = o0 + nb_full * 128
                last_n = NPC - (NW - 1) * WIN
                nc.sync.dma_start(out=bounceB[o1:o1 + last_n, :],
                                  in_=rm_buf[0:last_n, NW - 1, :])

            def all_gather_a(s):
                nc.gpsimd.collective_compute(
                    "AllGather", mybir.AluOpType.bypass,
                    ins=[bounceA.opt()], outs=[h_fullA[s].opt()],
                    replica_groups=[list(range(N_CORES))])

            def all_gather_b(s):
                nc.gpsimd.collective_compute(
                    "AllGather", mybir.AluOpType.bypass,
                    ins=[bounceB.opt()], outs=[h_fullB[s].opt()],
                    replica_groups=[list(range(N_CORES))])

            qrr = [0]

            def emit_gathers(gbuf, src_view, idx_tile, base_tile, n_tiles):
                for s0 in range(0, n_tiles, GT):
                    sn = min(GT, n_tiles - s0)
                    nc.gpsimd.dma_gather(
                        gbuf[:, s0:s0 + sn, :], src_view,
                        idx_tile[:, (base_tile + s0) * 8:
                                 (base_tile + s0 + sn) * 8],
                        num_idxs=sn * 128, num_idxs_reg=sn * 128,
                        elem_size=128, single_packet=True,
                        queue_num=qrr[0] % 4)
                    qrr[0] += 1

            # ---- input projection (chunk-pipelined epilogue) ----
            for j, (c0, cw) in enumerate(w_chunks):
                n0, cn = c0 * WIN, min(CW_N, NPC - c0 * WIN)
                xb = xpool.tile([IN_DIM, CW_N], f32r, name="xb")
                nc.sync.dma_start(out=xb[:, :cn],
                                  in_=xT_in[:, n0:n0 + cn].bitcast(f32r))
                ps = ps_big.tile([HID, CW_N], f32, name="psb")
                nc.tensor.matmul(ps[:, :cn], w_inT[:], xb[:, :cn],
                                 start=True, stop=True)
                nc.scalar.add(h_state[0][:, n0:n0 + cn], ps[:, :cn], b_in[:])
                transpose_windows(0, c0, c0 + cw)
                if j == BOUNCE_A_CHUNK:
                    bounce_a()
                    all_gather_a(0)
            bounce_b()
            all_gather_b(0)

            # ---- GIN layers ----
            for l in range(N_LAYERS):
                tblA = h_fullA[l][:]
                tblB = h_fullB[l][:]
                for j, (c0, cw) in enumerate(w_chunks):
                    g_a = ga_pool.tile([128, CHUNK_W * T_a, 128], f16,
                                       name="g_a")
                    emit_gathers(g_a, tblA, idx_a, c0 * T_a, cw * T_a)
                    g_b = gb_pool.tile([128, CHUNK_W * T_b, 128], f16,
                                       name="g_b")
                    emit_gathers(g_b, tblB, idx_b, c0 * T_b, cw * T_b)
                    hp = hp_pool.tile([HID, CW_N], f32r, name="hp")
                    for wl in range(cw):
                        w_i = c0 + wl
                        n0 = w_i * 128
                        wn = min(128, NPC - n0)
                        oh = oh_pool.tile([128, WIN, T], f16, name="oh")
                        nc.vector.tensor_tensor(
                            oh[:],
                            iota_wt[:],
                            dstloc[:, w_i, :].unsqueeze(1)
                                .broadcast_to([128, WIN, T]),
                            mybir.AluOpType.is_equal)
                        ps = ps_agg.tile([HID, WIN], f32, name="psa")
                        for t in range(T):
                            if t < T_a:
                                lhsT = g_a[:, wl * T_a + t, 0:HID]
                            else:
                                lhsT = g_b[:, wl * T_b + (t - T_a), 0:HID]
                            nc.tensor.matmul(ps[:], lhsT, oh[:, :, t],
                                             start=(t == 0),
                                             stop=(t == T - 1))
                        nc.vector.tensor_tensor(
                            hp[:, wl * WIN:wl * WIN + wn], ps[:, :wn],
                            h_state[l][:, n0:n0 + wn], mybir.AluOpType.add)
                    n0, cn = c0 * WIN, min(CW_N, NPC - c0 * WIN)
                    ps2 = ps_big.tile([HID, CW_N], f32, name="psb")
                    nc.tensor.matmul(ps2[:, :cn], w_lT[l][:], hp[:, :cn],
                                     start=True, stop=True)
                    nc.scalar.add(h_state[l + 1][:, n0:n0 + cn], ps2[:, :cn],
                                  b_l[l][:])
                    if l < N_LAYERS - 1:
                        transpose_windows(l + 1, c0, c0 + cw)
                        if j == BOUNCE_A_CHUNK:
                            bounce_a()
                        # AG-A mid-stream: its input (bounceA) is ready by
                        # the time GpSimd drains to here, so the flight
                        # overlaps the rest of this layer's gathers and
                        # next-layer A gathers can issue with no bubble
                        if j == BOUNCE_A_CHUNK + 3:
                            all_gather_a(l + 1)
                    else:
                        # interleave the output projection into layer 2
                        pso = ps_big.tile([OUT_DIM, CW_N], f32, name="pso",
                                          tag="psb")
                        for s in range(N_LAYERS + 1):
                            nc.tensor.matmul(pso[:, :cn], w_out4[s][:],
                                             h_state[s][:, n0:n0 + cn],
                                             start=(s == 0),
                                             stop=(s == N_LAYERS))
                        ot = ot_pool.tile([OUT_DIM, CW_N], f32, name="ot")
                        nc.scalar.add(ot[:, :cn], pso[:, :cn], b_out[:])
                        for tt in range(-(-cn // 128)):
                            t0 = tt * 128
                            tn = min(128, cn - t0)
                            pst = ps_tr.tile([128, 128], f32, name="psto",
                                             tag="pst")
                            nc.tensor.transpose(pst[:tn, :],
                                                ot[:, t0:t0 + tn], id128[:])
                            orow = ot_pool.tile([128, 128], f32, name="orow")
                            nc.scalar.copy(orow[:tn, :], pst[:tn, :])
                            nc.sync.dma_start(
                                out=out_ext[n0 + t0:n0 + t0 + tn, :],
                                in_=orow[:tn, :])
                if l < N_LAYERS - 1:
                    bounce_b()
                    all_gather_b(l + 1)

    nc.compile()
    return nc


def _get_nc_and_inputs(inputs):
    from concourse import bass_utils  # noqa: F401  (path setup)

    x = np.asarray(inputs["x"], np.float32)
    edge_index = np.asarray(inputs["edge_index"], np.int32)
    W_in = np.asarray(inputs["W_in"], np.float32)
    b_in = np.asarray(inputs["b_in"], np.float32)
    W_layers = np.asarray(inputs["W_layers"], np.float32)
    b_layers = np.asarray(inputs["b_layers"], np.float32)
    W_out = np.asarray(inputs["W_out"], np.float32)
    b_out = np.asarray(inputs["b_out"], np.float32)

    idx_a_w, idx_b_w, dstloc, T_a, T_b, perm = _prep(edge_index)

    key = ("nc", T_a, T_b)
    if key not in _cache:
        _cache.clear()
        _cache[key] = _build(T_a, T_b)
    nc = _cache[key]

    T = T_a + T_b
    inv = np.empty(N_NODES, np.int64)
    inv[perm] = np.arange(N_NODES)
    xT = np.ascontiguousarray(x.T[:, inv])
    w_inT = np.ascontiguousarray(W_in.T)
    w_lT = np.ascontiguousarray(W_layers.transpose(0, 2, 1))
    b_l = np.ascontiguousarray(b_layers[:, :, None])
    w_out4 = np.ascontiguousarray(
        np.stack([W_out[:, s * HID:(s + 1) * HID].T
                  for s in range(N_LAYERS + 1)])).astype(np.float16)
    iota_wt = np.ascontiguousarray(np.broadcast_to(
        np.arange(WIN, dtype=np.float16)[None, :, None],
        (128, WIN, T)))
    id96 = np.eye(HID, dtype=np.float16)
    id128 = np.eye(128, dtype=np.float32)

    in_maps = []
    for c in range(N_CORES):
        in_maps.append({
            "xT": np.ascontiguousarray(xT[:, c * NPC:(c + 1) * NPC]),
            "w_inT": w_inT,
            "b_in": b_in.reshape(HID, 1),
            "w_lT": w_lT,
            "b_l": b_l,
            "w_out4": w_out4,
            "b_out": b_out.reshape(OUT_DIM, 1),
            "iota_wt": iota_wt,
            "id96": id96,
            "id128": id128,
            "idx_a": idx_a_w[c],
            "idx_b": idx_b_w[c],
            "dstloc": dstloc[c],
        })
    return nc, in_maps, perm


def run(inputs, trace=False):
    from concourse import bass_utils

    nc, in_maps, perm = _get_nc_and_inputs(inputs)
    res = bass_utils.run_bass_kernel_spmd(
        nc, in_maps, core_ids=list(range(N_CORES)), trace=trace)
    out = np.concatenate([res.results[c]["out"] for c in range(N_CORES)], 0)
    return out[perm], res


def kernel(**inputs):
    out, _ = run(inputs, trace=False)
    return out


# revision 57
# speedup vs baseline: 1.0046x; 1.0046x over previous
"""GIN-style 3-layer GNN encoder on 8 Trainium2 NeuronCores (Bass/Tile).

Reference computation (fp32):
    h = x @ W_in.T + b_in                                  [50000, 96]
    for l in 0..2:
        agg = segment_sum(h[src], dst, N)                  [50000, 96]
        h = (h + agg) @ W_layers[l].T + b_layers[l]
    out = concat([h0..h3], 1) @ W_out.T + b_out            [50000, 128]

Distribution: nodes are partitioned across the 8 cores (6250/core) via a
host-side balancing permutation; each edge is owned by the core that owns
its dst node.  Each layer the updated node features are AllGathered into
two replicated row-major fp16 tables h_fullA/h_fullB (first/second half
of every core's node range, 25000 x 256B rows each, one Shared-space
pair per state so collectives write once and never alias) — the split
halves the AllGather latency on the critical path and keeps gather
indices < 32768 (int16).

Per-core segment sum: a core's node range is split into 49 windows of 128
nodes.  Every window has a fixed number of 128-edge tiles (T_a tiles with
src in half A, T_b in half B; the balancing permutation equalizes
per-window per-class edge counts so the fixed tile counts are tight).
Edge features are fetched with gpsimd dma_gather (fp16 256B rows, 1024
idxs per instruction, round-robin over the 4 SWDGE queues).  For each
window the one-hot onehot[e, j, t] = (j == dst_local[e, t]) is built on
DVE (layout [128, WIN, T] keeps every operand's last dim stride-1 so the
2x DVE perf mode engages), and the PE accumulates
    psum[96, 128] += gathered_tile[128e, 96].T @ onehot[:, :, t]
which is aggT for the window.

The whole layer is chunk-pipelined: each chunk of 4 windows (512 nodes =
one PSUM bank) flows gathers -> onehot/agg -> +h (DVE) -> layer matmul ->
bias -> PE transpose to the row-major fp16 shard.  bounceA DMAs fire
mid-layer (after window 24) and AllGather-A for the next table is emitted
mid-gather-stream (chunk 9) so its flight overlaps the rest of the layer;
AllGather-B fires at layer end (a collective's sequencer wait only blocks
until its input is ready — the flight itself runs async on the CC cores).
The final output projection is interleaved into layer 2's chunk loop.
"""
import sys

sys.path.insert(0, "/opt/trn_rl_repo")

import numpy as np

N_NODES = 50000
N_EDGES = 800000
IN_DIM = 128
HID = 96
OUT_DIM = 128
N_LAYERS = 3
N_CORES = 8
NPC = N_NODES // N_CORES          # 6250 nodes per core
WIN = 128                         # window width (nodes)
NW = (NPC + WIN - 1) // WIN       # 49 windows per core (last = 106 nodes)
HALF = NPC // 2                   # 3125: per-core A/B split
CLS = N_CORES * HALF              # 25000: A-class size
AW = HALF // WIN                  # 24 full-A windows per core
REM_A = HALF - AW * WIN           # 53 A-slots in window 24
CHUNK_W = 4                       # windows per chunk (= 512 nodes = 1 bank)
GT = 8                            # tiles per dma_gather (1024 idxs)
CW_N = CHUNK_W * WIN              # 512: node-chunk for dense matmuls
BOUNCE_A_CHUNK = (AW * WIN + REM_A - 1) // CW_N   # chunk whose transposes
                                                  # complete the A half (6)

_cache = {}


def _balance_nodes(src0, dst0):
    """Permute node ids so per-(core,window) A/B edge counts are even.

    A node's A/B class (which replicated gather table its row lives in) is
    frozen to its OLD id (< CLS -> A); the permutation only moves nodes
    within their class region, so per-node (deg_a, deg_b) are fixed and a
    greedy 2-D balance over the 392 (core, window) bins makes the uniform
    tile counts T_a/T_b tight.  Returns perm (old id -> new id).
    """
    deg_a = np.bincount(dst0[src0 < CLS], minlength=N_NODES).astype(np.int64)
    deg_b = np.bincount(dst0[src0 >= CLS], minlength=N_NODES).astype(np.int64)
    nbins = N_CORES * NW
    base = np.empty(nbins, np.int64)
    cap = np.empty(nbins, np.int64)
    for b in range(nbins):
        c, w = divmod(b, NW)
        base[b] = c * NPC + w * WIN
        cap[b] = min(WIN, NPC - w * WIN)
    woff = base % NPC
    q_a = np.clip(HALF - woff, 0, cap)   # A slots = first q_a of the window
    q_b = cap - q_a

    mu_a = max(1.0, deg_a.sum() / nbins)
    mu_b = max(1.0, deg_b.sum() / nbins)
    order = np.argsort(-(deg_a + deg_b), kind="stable")
    a_load = np.zeros(nbins)
    b_load = np.zeros(nbins)
    a_left = q_a.copy()
    b_left = q_b.copy()
    a_pos = np.zeros(nbins, np.int64)
    b_pos = q_a.copy()
    perm = np.empty(N_NODES, np.int64)
    for n in order:
        phi = np.maximum((a_load + deg_a[n]) / mu_a,
                         (b_load + deg_b[n]) / mu_b)
        if n < CLS:
            phi = np.where(a_left > 0, phi, np.inf)
            b_ = int(np.argmin(phi))
            perm[n] = base[b_] + a_pos[b_]
            a_pos[b_] += 1
            a_left[b_] -= 1
        else:
            phi = np.where(b_left > 0, phi, np.inf)
            b_ = int(np.argmin(phi))
            perm[n] = base[b_] + b_pos[b_]
            b_pos[b_] += 1
            b_left[b_] -= 1
        a_load[b_] += deg_a[n]
        b_load[b_] += deg_b[n]
    return perm


def _prep(edge_index):
    """Host-side edge bucketing -> per-core gather index / dst tables."""
    src0 = edge_index[0].astype(np.int64)
    dst0 = edge_index[1].astype(np.int64)
    perm = _balance_nodes(src0, dst0)
    src = perm[src0]
    dst = perm[dst0]
    core = dst // NPC
    din = dst % NPC
    w = din // WIN
    dstl = din % WIN
    s_in = src % NPC
    c_src = src // NPC
    is_b = (s_in >= HALF).astype(np.int64)
    pos = np.where(is_b == 0, c_src * HALF + s_in,
                   c_src * HALF + s_in - HALF)  # < 25000, int16-safe

    key = (core * NW + w) * 2 + is_b
    order = np.argsort(key, kind="stable")
    s_pos = pos[order]
    s_dstl = dstl[order]
    s_key = key[order]
    s_b = is_b[order]

    counts = np.bincount(key, minlength=N_CORES * NW * 2)
    T_a = max(1, int(-(-counts.reshape(-1, 2)[:, 0].max() // 128)))
    T_b = max(1, int(-(-counts.reshape(-1, 2)[:, 1].max() // 128)))
    T = T_a + T_b

    starts = np.zeros(N_CORES * NW * 2, np.int64)
    starts[1:] = np.cumsum(counts)[:-1]
    rank = np.arange(len(s_key)) - starts[s_key]

    c_arr = s_key // (2 * NW)
    w_arr = (s_key // 2) % NW

    idx_a = np.zeros((N_CORES, NW, T_a * 128), np.int16)
    idx_b = np.zeros((N_CORES, NW, T_b * 128), np.int16)
    dstl_arr = np.full((N_CORES, NW, T, 128), -1.0, np.float16)

    a_m = s_b == 0
    flat = (c_arr[a_m] * NW + w_arr[a_m]) * (T_a * 128) + rank[a_m]
    idx_a.reshape(-1)[flat] = s_pos[a_m].astype(np.int16)
    t_g = rank[a_m] // 128
    e_g = rank[a_m] % 128
    flat = ((c_arr[a_m] * NW + w_arr[a_m]) * T + t_g) * 128 + e_g
    dstl_arr.reshape(-1)[flat] = s_dstl[a_m].astype(np.float16)

    b_m = ~a_m
    flat = (c_arr[b_m] * NW + w_arr[b_m]) * (T_b * 128) + rank[b_m]
    idx_b.reshape(-1)[flat] = s_pos[b_m].astype(np.int16)
    t_g = rank[b_m] // 128 + T_a
    e_g = rank[b_m] % 128
    flat = ((c_arr[b_m] * NW + w_arr[b_m]) * T + t_g) * 128 + e_g
    dstl_arr.reshape(-1)[flat] = s_dstl[b_m].astype(np.float16)

    def wrap(vals):  # [NW*Tc*128] -> [128, NW*Tc*8] int16 wrapped+replicated
        v = vals.reshape(-1, 16).T
        return np.tile(v, (8, 1)).copy()

    idx_a_w = np.stack([wrap(idx_a[c].reshape(-1)) for c in range(N_CORES)])
    idx_b_w = np.stack([wrap(idx_b[c].reshape(-1)) for c in range(N_CORES)])
    dstloc = np.ascontiguousarray(dstl_arr.transpose(0, 3, 1, 2))  # [C,128,NW,T]
    return idx_a_w, idx_b_w, dstloc, T_a, T_b, perm


def _build(T_a, T_b):
    from concourse import bacc, tile, mybir, library_config

    dt = mybir.dt
    T = T_a + T_b
    nc = bacc.Bacc("TRN2", target_bir_lowering=False, debug=False,
                   num_devices=N_CORES, num_swdge_queues=4,
                   dynamic_dma_scratch_size=49152)

    # ---- I/O ----
    xT_in = nc.dram_tensor("xT", [IN_DIM, NPC], dt.float32, kind="ExternalInput")
    w_inT_in = nc.dram_tensor("w_inT", [IN_DIM, HID], dt.float32,
                              kind="ExternalInput")
    b_in_in = nc.dram_tensor("b_in", [HID, 1], dt.float32, kind="ExternalInput")
    w_lT_in = nc.dram_tensor("w_lT", [N_LAYERS, HID, HID], dt.float32,
                             kind="ExternalInput")
    b_l_in = nc.dram_tensor("b_l", [N_LAYERS, HID, 1], dt.float32,
                            kind="ExternalInput")
    w_out4_in = nc.dram_tensor("w_out4", [N_LAYERS + 1, HID, OUT_DIM],
                               dt.float16, kind="ExternalInput")
    b_out_in = nc.dram_tensor("b_out", [OUT_DIM, 1], dt.float32,
                              kind="ExternalInput")
    iota_in = nc.dram_tensor("iota_wt", [128, WIN, T], dt.float16,
                             kind="ExternalInput")
    id96_in = nc.dram_tensor("id96", [HID, HID], dt.float16,
                             kind="ExternalInput")
    id128_in = nc.dram_tensor("id128", [128, 128], dt.float32,
                              kind="ExternalInput")
    idx_a_in = nc.dram_tensor("idx_a", [128, NW * T_a * 8], dt.int16,
                              kind="ExternalInput")
    idx_b_in = nc.dram_tensor("idx_b", [128, NW * T_b * 8], dt.int16,
                              kind="ExternalInput")
    dstloc_in = nc.dram_tensor("dstloc", [128, NW, T], dt.float16,
                               kind="ExternalInput")
    out_ext = nc.dram_tensor("out", [NPC, OUT_DIM], dt.float32,
                             kind="ExternalOutput")

    f32, f32r, f16 = dt.float32, dt.float32r, dt.float16

    with tile.TileContext(nc, num_cores=N_CORES) as tc:
        nc.gpsimd.load_library(library_config.mlp)
        with tc.tile_pool(name="persist", bufs=1) as pp, \
             tc.tile_pool(name="xpool", bufs=2) as xpool, \
             tc.tile_pool(name="hp", bufs=3) as hp_pool, \
             tc.tile_pool(name="ga", bufs=3) as ga_pool, \
             tc.tile_pool(name="gb", bufs=3) as gb_pool, \
             tc.tile_pool(name="oh", bufs=3) as oh_pool, \
             tc.tile_pool(name="otile", bufs=2) as ot_pool, \
             tc.tile_pool(name="ps_agg", bufs=4, space="PSUM") as ps_agg, \
             tc.tile_pool(name="ps_big", bufs=2, space="PSUM") as ps_big, \
             tc.tile_pool(name="ps_tr", bufs=2, space="PSUM") as ps_tr, \
             tc.tile_pool(name="dram", bufs=1, space="DRAM") as dram:

            def load(name, shape, dtype, src_ap):
                t = pp.tile(shape, dtype, name=name)
                nc.sync.dma_start(out=t[:], in_=src_ap)
                return t

            w_inT = load("w_inT", [IN_DIM, HID], f32r, w_inT_in[:].bitcast(f32r))
            b_in = load("b_in", [HID, 1], f32, b_in_in[:])
            w_lT = [load(f"w_lT{l}", [HID, HID], f32r, w_lT_in[l].bitcast(f32r))
                    for l in range(N_LAYERS)]
            b_l = [load(f"b_l{l}", [HID, 1], f32, b_l_in[l])
                   for l in range(N_LAYERS)]
            w_out4 = [load(f"w_out4_{s}", [HID, OUT_DIM], f16, w_out4_in[s])
                      for s in range(N_LAYERS + 1)]
            b_out = load("b_out", [OUT_DIM, 1], f32, b_out_in[:])
            iota_wt = load("iota_wt", [128, WIN, T], f16, iota_in[:])
            id96 = load("id96", [HID, HID], f16, id96_in[:])
            id128 = load("id128", [128, 128], f32, id128_in[:])
            idx_a = load("idx_a", [128, NW * T_a * 8], dt.int16, idx_a_in[:])
            idx_b = load("idx_b", [128, NW * T_b * 8], dt.int16, idx_b_in[:])
            dstloc = load("dstloc", [128, NW, T], f16, dstloc_in[:])

            h_state = [pp.tile([HID, NPC], f16, name=f"h{s}")
                       for s in range(N_LAYERS + 1)]
            rm_buf = pp.tile([128, NW, 128], f16, name="rm_buf")

            # double-buffered replicated tables: state s lives in buf s%2
            h_fullA = [dram.tile([CLS, 128], f16, name=f"h_fullA{i}",
                                 addr_space="Shared")
                       for i in range(N_LAYERS)]
            h_fullB = [dram.tile([N_NODES - CLS, 128], f16, name=f"h_fullB{i}",
                                 addr_space="Shared")
                       for i in range(N_LAYERS)]
            bounceA = dram.tile([HALF, 128], f16)
            bounceB = dram.tile([NPC - HALF, 128], f16)

            w_chunks = [(c0, min(CHUNK_W, NW - c0))
                        for c0 in range(0, NW, CHUNK_W)]

            def transpose_windows(s, w0, w1):
                for t in range(w0, w1):
                    n0 = t * 128
                    tn = min(128, NPC - n0)
                    pst = ps_tr.tile([128, HID], f16, name="pst")
                    nc.tensor.transpose(pst[:tn, :],
                                        h_state[s][:, n0:n0 + tn], id96[:])
                    nc.scalar.copy(rm_buf[:tn, t, 0:HID], pst[:tn, :])

            def bounce_a():
                nc.sync.dma_start(
                    out=bounceA[0:AW * 128, :].rearrange(
                        "(t p) d -> p t d", p=128),
                    in_=rm_buf[:, 0:AW, :])
                nc.sync.dma_start(out=bounceA[AW * 128:HALF, :],
                                  in_=rm_buf[0:REM_A, AW, :])

            def bounce_b():
                nc.sync.dma_start(out=bounceB[0:WIN - REM_A, :],
                                  in_=rm_buf[REM_A:WIN, AW, :])
                nb_full = NW - AW - 2   # full windows AW+1 .. NW-2
                o0 = WIN - REM_A
                nc.sync.dma_start(
                    out=bounceB[o0:o0 + nb_full * 128, :].rearrange(
                        "(t p) d -> p t d", p=128),
                    in_=rm_buf[:, AW + 1:NW - 1, :])
                o1 = o0 + nb_full * 128
                last_n = NPC - (NW - 1) * WIN
                nc.sync.dma_start(out=bounceB[o1:o1 + last_n, :],
                                  in_=rm_buf[0:last_n, NW - 1, :])

            def all_gather_a(s):
                nc.gpsimd.collective_compute(
                    "AllGather", mybir.AluOpType.bypass,
                    ins=[bounceA.opt()], outs=[h_fullA[s].opt()],
                    replica_groups=[list(range(N_CORES))])

            def all_gather_b(s):
                nc.gpsimd.collective_compute(
                    "AllGather", mybir.AluOpType.bypass,
                    ins=[bounceB.opt()], outs=[h_fullB[s].opt()],
                    replica_groups=[list(range(N_CORES))])

            qrr = [0]

            def emit_gathers(gbuf, src_view, idx_tile, base_tile, n_tiles):
                for s0 in range(0, n_tiles, GT):
                    sn = min(GT, n_tiles - s0)
                    nc.gpsimd.dma_gather(
                        gbuf[:, s0:s0 + sn, :], src_view,
                        idx_tile[:, (base_tile + s0) * 8:
                                 (base_tile + s0 + sn) * 8],
                        num_idxs=sn * 128, num_idxs_reg=sn * 128,
                        elem_size=128, single_packet=True,
                        queue_num=qrr[0] % 4)
                    qrr[0] += 1

            # ---- input projection (chunk-pipelined epilogue) ----
            for j, (c0, cw) in enumerate(w_chunks):
                n0, cn = c0 * WIN, min(CW_N, NPC - c0 * WIN)
                xb = xpool.tile([IN_DIM, CW_N], f32r, name="xb")
                nc.sync.dma_start(out=xb[:, :cn],
                                  in_=xT_in[:, n0:n0 + cn].bitcast(f32r))
                ps = ps_big.tile([HID, CW_N], f32, name="psb")
                nc.tensor.matmul(ps[:, :cn], w_inT[:], xb[:, :cn],
                                 start=True, stop=True)
                nc.scalar.add(h_state[0][:, n0:n0 + cn], ps[:, :cn], b_in[:])
                transpose_windows(0, c0, c0 + cw)
                if j == BOUNCE_A_CHUNK:
                    bounce_a()
                    all_gather_a(0)
            bounce_b()
            all_gather_b(0)

            # ---- GIN layers ----
            for l in range(N_LAYERS):
                tblA = h_fullA[l][:]
                tblB = h_fullB[l][:]
                for j, (c0, cw) in enumerate(w_chunks):
                    g_a = ga_pool.tile([128, CHUNK_W * T_a, 128], f16,
                                       name="g_a")
                    emit_gathers(g_a, tblA, idx_a, c0 * T_a, cw * T_a)
                    g_b = gb_pool.tile([128, CHUNK_W * T_b, 128], f16,
                                       name="g_b")
                    emit_gathers(g_b, tblB, idx_b, c0 * T_b, cw * T_b)
                    hp = hp_pool.tile([HID, CW_N], f32r, name="hp")
                    for wl in range(cw):
                        w_i = c0 + wl
                        n0 = w_i * 128
                        wn = min(128, NPC - n0)
                        oh = oh_pool.tile([128, WIN, T], f16, name="oh")
                        nc.vector.tensor_tensor(
                            oh[:],
                            iota_wt[:],
                            dstloc[:, w_i, :].unsqueeze(1)
                                .broadcast_to([128, WIN, T]),
                            mybir.AluOpType.is_equal)
                        ps = ps_agg.tile([HID, WIN], f32, name="psa")
                        for t in range(T):
                            if t < T_a:
                                lhsT = g_a[:, wl * T_a + t, 0:HID]
                            else:
                                lhsT = g_b[:, wl * T_b + (t - T_a), 0:HID]
                            nc.tensor.matmul(ps[:], lhsT, oh[:, :, t],
                                             start=(t == 0),
                                             stop=(t == T - 1))
                        nc.vector.tensor_tensor(
                            hp[:, wl * WIN:wl * WIN + wn], ps[:, :wn],
                            h_state[l][:, n0:n0 + wn], mybir.AluOpType.add)
                    n0, cn = c0 * WIN, min(CW_N, NPC - c0 * WIN)
                    ps2 = ps_big.tile([HID, CW_N], f32, name="psb")
                    nc.tensor.matmul(ps2[:, :cn], w_lT[l][:], hp[:, :cn],
                                     start=True, stop=True)
                    nc.scalar.add(h_state[l + 1][:, n0:n0 + cn], ps2[:, :cn],
                                  b_l[l][:])
                    if l < N_LAYERS - 1:
                        transpose_windows(l + 1, c0, c0 + cw)
                        if j == BOUNCE_A_CHUNK:
                            bounce_a()
                        # AG-A mid-stream: its input (bounceA) is ready by
                        # the time GpSimd drains to here, so the flight
                        # overlaps the rest of this layer's gathers and
                        # next-layer A gathers can issue with no bubble
                        if j == BOUNCE_A_CHUNK + 3:
                            all_gather_a(l + 1)
                    else:
                        # interleave the output projection into layer 2
                        pso = ps_big.tile([OUT_DIM, CW_N], f32, name="pso",
                                          tag="psb")
                        for s in range(N_LAYERS + 1):
                            nc.tensor.matmul(pso[:, :cn], w_out4[s][:],
                                             h_state[s][:, n0:n0 + cn],
                                             start=(s == 0),
                                             stop=(s == N_LAYERS))
                        ot = ot_pool.tile([OUT_DIM, CW_N], f32, name="ot")
                        nc.scalar.add(ot[:, :cn], pso[:, :cn], b_out[:])
                        for tt in range(-(-cn // 128)):
                            t0 = tt * 128
                            tn = min(128, cn - t0)
                            pst = ps_tr.tile([128, 128], f32, name="psto",
                                             tag="pst")
                            nc.tensor.transpose(pst[:tn, :],
                                                ot[:, t0:t0 + tn], id128[:])
                            orow = ot_pool.tile([128, 128], f32, name="orow")
                            nc.scalar.copy(orow[:tn, :], pst[:tn, :])
                            nc.sync.dma_start(
                                out=out_ext[n0 + t0:n0 + t0 + tn, :],
                                in_=orow[:tn, :])
                if l < N_LAYERS - 1:
                    bounce_b()
                    all_gather_b(l + 1)

    nc.compile()
    return nc


def _get_nc_and_inputs(inputs):
    from concourse import bass_utils  # noqa: F401  (path setup)

    x = np.asarray(inputs["x"], np.float32)
    edge_index = np.asarray(inputs["edge_index"], np.int32)
    W_in = np.asarray(inputs["W_in"], np.float32)
    b_in = np.asarray(inputs["b_in"], np.float32)
    W_layers = np.asarray(inputs["W_layers"], np.float32)
    b_layers = np.asarray(inputs["b_layers"], np.float32)
    W_out = np.asarray(inputs["W_out"], np.float32)
    b_out = np.asarray(inputs["b_out"], np.float32)

    idx_a_w, idx_b_w, dstloc, T_a, T_b, perm = _prep(edge_index)

    key = ("nc", T_a, T_b)
    if key not in _cache:
        _cache.clear()
        _cache[key] = _build(T_a, T_b)
    nc = _cache[key]

    T = T_a + T_b
    inv = np.empty(N_NODES, np.int64)
    inv[perm] = np.arange(N_NODES)
    xT = np.ascontiguousarray(x.T[:, inv])
    w_inT = np.ascontiguousarray(W_in.T)
    w_lT = np.ascontiguousarray(W_layers.transpose(0, 2, 1))
    b_l = np.ascontiguousarray(b_layers[:, :, None])
    w_out4 = np.ascontiguousarray(
        np.stack([W_out[:, s * HID:(s + 1) * HID].T
                  for s in range(N_LAYERS + 1)])).astype(np.float16)
    iota_wt = np.ascontiguousarray(np.broadcast_to(
        np.arange(WIN, dtype=np.float16)[None, :, None],
        (128, WIN, T)))
    id96 = np.eye(HID, dtype=np.float16)
    id128 = np.eye(128, dtype=np.float32)

    in_maps = []
    for c in range(N_CORES):
        in_maps.append({
            "xT": np.ascontiguousarray(xT[:, c * NPC:(c + 1) * NPC]),
            "w_inT": w_inT,
            "b_in": b_in.reshape(HID, 1),
            "w_lT": w_lT,
            "b_l": b_l,
            "w_out4": w_out4,
            "b_out": b_out.reshape(OUT_DIM, 1),
            "iota_wt": iota_wt,
            "id96": id96,
            "id128": id128,
            "idx_a": idx_a_w[c],
            "idx_b": idx_b_w[c],
            "dstloc": dstloc[c],
        })
    return nc, in_maps, perm


def run(inputs, trace=False):
    from concourse import bass_utils

    nc, in_maps, perm = _get_nc_and_inputs(inputs)
    res = bass_utils.run_bass_kernel_spmd(
        nc, in_maps, core_ids=list(range(N_CORES)), trace=trace)
    out = np.concatenate([res.results[c]["out"] for c in range(N_CORES)], 0)
    return out[perm], res


def kernel(**inputs):
    out, _ = run(inputs, trace=False)
    return out
